# revision 10
# baseline (speedup 1.0000x reference)
"""Expert-parallel MoE GEGLU MLP (RMSNorm -> c_fc -> GEGLU -> c_proj) on 8
Trainium2 NeuronCores.

Sharding: expert-parallel. Core e computes the full MLP for expert e's tokens
(x[:, e] -> [8192, 768]); no collectives. gamma*sqrt(D) is folded into c_fc
and mult_bias into c_proj on the host, so the device kernel computes:

    h   = x / ||x||_2            (per token, fp32 accumulate)
    u   = h @ W1                 (bf16 x bf16 -> fp32 PSUM)
    g   = gelu(u_gate) * u_val   (exact erf gelu on ACT)
    out = g @ W2                 (bf16 x bf16 -> fp32 PSUM)

Layout: tokens stream in super-blocks of 1024. x is loaded twice: once
token-major (for the squared-sum only) and once d-major via the DMA xbar
transpose straight from DRAM. The per-token rsqrt scale is computed
token-major (cheap DVE Newton), moved to a row with one tiny PE transpose,
broadcast across partitions with K=1 matmuls, and applied in place to the
transposed activations. GEMM1 runs with hidden on PSUM partitions and
1024-token moving operands; GEMM2 uses the GEGLU output chunks as the
stationary operand so its PSUM output is already token-major - no output
transposes at all.
"""

from contextlib import ExitStack

import ml_dtypes
import numpy as np

import concourse.bass as bass
import concourse.mybir as mybir
import concourse.tile as tile
from concourse import bacc
from concourse.bass_utils import run_bass_kernel_spmd
from concourse.masks import make_identity

# Problem dims (fixed by the nn_MLP_90795608637901 spec).
B, E, CAP, D = 8, 8, 1024, 768
H = 2048
H2 = 2 * H
T = B * CAP          # tokens per expert (per core) = 8192
SB = 1024            # tokens per super-block
NSB = T // SB        # 8
S = SB // 128        # 8 partition sub-tiles per super-block
KC1 = D // 128       # 6 contraction chunks for GEMM1
MC = H // 128        # 16 value/gate chunk pairs
KC2 = H // 128       # 16 contraction chunks for GEMM2

BF = mybir.dt.bfloat16
F32 = mybir.dt.float32
I32 = mybir.dt.int32
ALU = mybir.AluOpType


def build_kernel(nsb: int = NSB) -> bass.Bass:
    nc = bacc.Bacc("TRN2", target_bir_lowering=False, debug=False)

    t = nsb * SB
    x = nc.declare_dram_parameter("x", [t, D], BF, isOutput=False)
    xT = nc.declare_dram_parameter("xT", [D, t], BF, isOutput=False)
    w1 = nc.declare_dram_parameter("w1", [D, H2], BF, isOutput=False)
    w2 = nc.declare_dram_parameter("w2", [H, D], BF, isOutput=False)
    sel = nc.declare_dram_parameter("sel", [S, SB], F32, isOutput=False)
    out = nc.declare_dram_parameter("out", [t, D], BF, isOutput=True)

    with tile.TileContext(nc) as tc, ExitStack() as ctx:
        weights = ctx.enter_context(tc.tile_pool(name="weights", bufs=1))
        io_in = ctx.enter_context(tc.tile_pool(name="io_in", bufs=2))
        work = ctx.enter_context(tc.tile_pool(name="work", bufs=2))
        gpool = ctx.enter_context(tc.tile_pool(name="gpool", bufs=1))
        small = ctx.enter_context(tc.tile_pool(name="small", bufs=2))
        agp = ctx.enter_context(tc.tile_pool(name="agp", bufs=3))
        obp = ctx.enter_context(tc.tile_pool(name="obp", bufs=3))
        psum_mm = ctx.enter_context(tc.tile_pool(name="psum_mm", bufs=4, space="PSUM"))
        psum_sc = ctx.enter_context(tc.tile_pool(name="psum_sc", bufs=1, space="PSUM"))
        psum_yt = ctx.enter_context(tc.tile_pool(name="psum_yt", bufs=2, space="PSUM"))

        # x DMAs for a super-block; emitted ahead of the weight loads for
        # sb=0 so the PE pipeline can start before 19MB of weights land.
        x_tiles = {}

        def issue_x(sb):
            xb = io_in.tile([128, S, D], BF, name="xb", tag="xb")
            xv = x[sb * SB:(sb + 1) * SB].rearrange("(s p) d -> p s d", p=128)
            nc.sync.dma_start(out=xb, in_=xv)
            xt = work.tile([128, KC1, SB], BF, name="xt", tag="xt")
            for k in range(KC1):
                nc.scalar.dma_start(
                    out=xt[:, k, :],
                    in_=xT[k * 128:(k + 1) * 128, sb * SB:(sb + 1) * SB],
                )
            x_tiles[sb] = (xb, xt)

        issue_x(0)

        # Resident weights: [p, k, n] with contraction index = k*128 + p.
        # W1 lands in (value-block, gate-block) column pairs so the first
        # GEMM1 chunks can start ~10us in instead of waiting for 12.6MB.
        w1s = weights.tile([128, KC1, H2], BF)
        for nb in range(4):
            for base in (0, H):
                c0, c1 = base + nb * 512, base + (nb + 1) * 512
                for k in range(KC1):
                    nc.sync.dma_start(out=w1s[:, k, c0:c1],
                                      in_=w1[k * 128:(k + 1) * 128, c0:c1])
        w2s = weights.tile([128, KC2, D], BF)
        for k in range(KC2):
            nc.sync.dma_start(out=w2s[:, k, :], in_=w2[k * 128:(k + 1) * 128, :])

        ident = weights.tile([128, 128], F32)
        make_identity(nc, ident)
        # sel[s, s*128+q] = 1: selector for the partition-broadcast matmul
        sels = weights.tile([S, SB], F32)
        nc.sync.dma_start(out=sels, in_=sel[:, :])
        bias0 = weights.tile([128, 1], F32)
        nc.vector.memset(bias0, 0.0)

        for sb in range(nsb):
            if sb + 1 < nsb:
                issue_x(sb + 1)
            xb, xt = x_tiles.pop(sb)

            # --- RMSNorm scale, token-major: ss on ACT, rsqrt on DVE ---
            ssb = small.tile([128, S], F32, name="ssb")
            sq = small.tile([128, D], BF, name="sq")
            for s in range(S):
                nc.scalar.activation(
                    sq, xb[:, s], mybir.ActivationFunctionType.Square,
                    bias=bias0, accum_out=ssb[:, s:s + 1],
                )
            yb = small.tile([128, S], F32, name="yb")
            tb = small.tile([128, S], F32, name="tb")
            # rsqrt seed via the int bit trick: 0x5f3759df - (i >> 1)
            # (written as (i>>1 xor -1) + 0x5f3759df + 1), then 3 Newton steps.
            nc.vector.tensor_scalar(
                out=yb.bitcast(I32), in0=ssb.bitcast(I32),
                scalar1=1, scalar2=-1,
                op0=ALU.logical_shift_right, op1=ALU.bitwise_xor,
            )
            nc.vector.tensor_scalar(
                out=yb.bitcast(I32), in0=yb.bitcast(I32),
                scalar1=0x5F375A60, scalar2=None, op0=ALU.add,
            )
            for _ in range(3):
                nc.vector.tensor_mul(tb, yb, yb)
                nc.vector.tensor_mul(tb, tb, ssb)
                nc.vector.tensor_scalar(
                    out=tb, in0=tb, scalar1=-0.5, scalar2=1.5,
                    op0=ALU.mult, op1=ALU.add,
                )
                nc.vector.tensor_mul(yb, yb, tb)

            # --- broadcast scale across partitions: yb[p,s] -> sc[:,s*128+p]
            yt = psum_yt.tile([S, 128], F32, name="yt", tag="yt", space="PSUM")
            nc.tensor.transpose(yt, yb, ident)
            yrow = small.tile([S, 128], F32, name="yrow")
            nc.vector.tensor_copy(yrow, yt)
            psc = psum_sc.tile([128, SB], F32, name="psc", tag="sc", space="PSUM")
            for s in range(S):
                nc.tensor.matmul(
                    psc[:, s * 128:(s + 1) * 128],
                    lhsT=sels[:, s * 128:(s + 1) * 128],
                    rhs=yrow, start=True, stop=True,
                )
            sc = work.tile([128, SB], F32, name="sc", tag="sc")
            nc.vector.tensor_copy(sc, psc)

            # --- normalize in place in the transposed domain ---
            for k in range(KC1):
                nc.vector.tensor_mul(xt[:, k, :], xt[:, k, :], sc)

            # --- GEMM1 + GEGLU, one value/gate chunk pair at a time.
            # A matmul's fp32 PSUM output cannot cross a 2KB bank, so the
            # 1024-token super-block runs as two 512-column halves. ---
            gbuf = gpool.tile([128, KC2, SB], BF, name="gbuf")
            for m in range(MC):
                for h2 in range(2):
                    cols = slice(h2 * 512, (h2 + 1) * 512)
                    pv = psum_mm.tile([128, 512], F32, name="pv", tag="mm",
                                      space="PSUM")
                    pg = psum_mm.tile([128, 512], F32, name="pg", tag="mm",
                                      space="PSUM")
                    for k in range(KC1):
                        nc.tensor.matmul(
                            pv, lhsT=w1s[:, k, m * 128:(m + 1) * 128],
                            rhs=xt[:, k, cols],
                            start=(k == 0), stop=(k == KC1 - 1),
                        )
                    for k in range(KC1):
                        nc.tensor.matmul(
                            pg, lhsT=w1s[:, k, H + m * 128:H + (m + 1) * 128],
                            rhs=xt[:, k, cols],
                            start=(k == 0), stop=(k == KC1 - 1),
                        )
                    ag = agp.tile([128, 512], F32, name="ag")
                    nc.scalar.activation(
                        ag, pg, mybir.ActivationFunctionType.Gelu, bias=bias0,
                    )
                    nc.vector.tensor_mul(gbuf[:, m, cols], pv, ag)

            # --- GEMM2 with gbuf chunks stationary: PSUM comes out
            # token-major, so results DMA straight out after one copy.
            # d=768 output splits into 512+256 PSUM chains (bank rule). ---
            for mt in range(S):
                ob = obp.tile([128, D], BF, name="ob")
                for d0, d1 in ((0, 512), (512, 768)):
                    po = psum_mm.tile([128, d1 - d0], F32, name="po", tag="mm",
                                      space="PSUM")
                    for k2 in range(KC2):
                        nc.tensor.matmul(
                            po, lhsT=gbuf[:, k2, mt * 128:(mt + 1) * 128],
                            rhs=w2s[:, k2, d0:d1],
                            start=(k2 == 0), stop=(k2 == KC2 - 1),
                        )
                    nc.vector.tensor_copy(ob[:, d0:d1], po)
                nc.gpsimd.dma_start(
                    out=out[sb * SB + mt * 128:sb * SB + (mt + 1) * 128, :],
                    in_=ob,
                )

    nc.finalize()
    return nc


def prepare_in_maps(x, c_fc, c_proj, gamma, mult_bias):
    bf16 = ml_dtypes.bfloat16
    g = (gamma.astype(np.float32) * np.float32(np.sqrt(D)))
    w1_all = (c_fc.astype(np.float32) * g[None, :, None]).astype(bf16)
    w2_all = (c_proj.astype(np.float32)
              * mult_bias.astype(np.float32)[None, :, None]).astype(bf16)
    xs = np.ascontiguousarray(np.transpose(x, (1, 0, 2, 3))).reshape(E, T, D)
    xs = xs.astype(bf16)
    xts = np.ascontiguousarray(np.transpose(xs, (0, 2, 1)))
    sel = np.zeros((S, SB), np.float32)
    for s in range(S):
        sel[s, s * 128:(s + 1) * 128] = 1.0
    return [
        {"x": xs[e], "xT": xts[e], "w1": w1_all[e], "w2": w2_all[e], "sel": sel}
        for e in range(E)
    ]


def run(in_maps, trace: bool = False):
    nc = build_kernel()
    return run_bass_kernel_spmd(
        nc, in_maps, core_ids=list(range(E)), trace=trace,
    )


def kernel(x, c_fc, c_proj, gamma, mult_bias):
    in_maps = prepare_in_maps(x, c_fc, c_proj, gamma, mult_bias)
    res = run(in_maps)
    out = np.empty((E, B, CAP, D), np.float32)
    for e in range(E):
        out[e] = res.results[e]["out"].astype(np.float32).reshape(B, CAP, D)
    return np.ascontiguousarray(out.transpose(1, 0, 2, 3))


# revision 11
# speedup vs baseline: 1.0041x; 1.0041x over previous
"""Expert-parallel MoE GEGLU MLP (RMSNorm -> c_fc -> GEGLU -> c_proj) on 8
Trainium2 NeuronCores.

Sharding: expert-parallel. Core e computes the full MLP for expert e's tokens
(x[:, e] -> [8192, 768]); no collectives. gamma*sqrt(D) is folded into c_fc
and mult_bias into c_proj on the host, so the device kernel computes:

    h   = x / ||x||_2            (per token, fp32 accumulate)
    u   = h @ W1                 (bf16 x bf16 -> fp32 PSUM)
    g   = gelu(u_gate) * u_val   (exact erf gelu on ACT)
    out = g @ W2                 (bf16 x bf16 -> fp32 PSUM)

Layout: tokens stream in super-blocks of 1024. x is loaded twice: once
token-major (for the squared-sum only) and once d-major via the DMA xbar
transpose straight from DRAM. The per-token rsqrt scale is computed
token-major (cheap DVE Newton), moved to a row with one tiny PE transpose,
broadcast across partitions with K=1 matmuls, and applied in place to the
transposed activations. GEMM1 runs with hidden on PSUM partitions and
1024-token moving operands; GEMM2 uses the GEGLU output chunks as the
stationary operand so its PSUM output is already token-major - no output
transposes at all.
"""

from contextlib import ExitStack

import ml_dtypes
import numpy as np

import concourse.bass as bass
import concourse.mybir as mybir
import concourse.tile as tile
from concourse import bacc
from concourse.bass_utils import run_bass_kernel_spmd
from concourse.masks import make_identity

# Problem dims (fixed by the nn_MLP_90795608637901 spec).
B, E, CAP, D = 8, 8, 1024, 768
H = 2048
H2 = 2 * H
T = B * CAP          # tokens per expert (per core) = 8192
SB = 1024            # tokens per super-block
NSB = T // SB        # 8
S = SB // 128        # 8 partition sub-tiles per super-block
KC1 = D // 128       # 6 contraction chunks for GEMM1
MC = H // 128        # 16 value/gate chunk pairs
KC2 = H // 128       # 16 contraction chunks for GEMM2

BF = mybir.dt.bfloat16
F32 = mybir.dt.float32
I32 = mybir.dt.int32
ALU = mybir.AluOpType


def build_kernel(nsb: int = NSB) -> bass.Bass:
    nc = bacc.Bacc("TRN2", target_bir_lowering=False, debug=False)

    t = nsb * SB
    x = nc.declare_dram_parameter("x", [t, D], BF, isOutput=False)
    xT = nc.declare_dram_parameter("xT", [D, t], BF, isOutput=False)
    w1 = nc.declare_dram_parameter("w1", [D, H2], BF, isOutput=False)
    w2 = nc.declare_dram_parameter("w2", [H, D], BF, isOutput=False)
    sel = nc.declare_dram_parameter("sel", [S, SB], F32, isOutput=False)
    out = nc.declare_dram_parameter("out", [t, D], BF, isOutput=True)

    with tile.TileContext(nc) as tc, ExitStack() as ctx:
        weights = ctx.enter_context(tc.tile_pool(name="weights", bufs=1))
        io_in = ctx.enter_context(tc.tile_pool(name="io_in", bufs=2))
        work = ctx.enter_context(tc.tile_pool(name="work", bufs=2))
        gpool = ctx.enter_context(tc.tile_pool(name="gpool", bufs=1))
        small = ctx.enter_context(tc.tile_pool(name="small", bufs=2))
        agp = ctx.enter_context(tc.tile_pool(name="agp", bufs=3))
        obp = ctx.enter_context(tc.tile_pool(name="obp", bufs=3))
        psum_mm = ctx.enter_context(tc.tile_pool(name="psum_mm", bufs=4, space="PSUM"))
        psum_sc = ctx.enter_context(tc.tile_pool(name="psum_sc", bufs=1, space="PSUM"))
        psum_yt = ctx.enter_context(tc.tile_pool(name="psum_yt", bufs=2, space="PSUM"))

        # x DMAs for a super-block; emitted ahead of the weight loads for
        # sb=0 so the PE pipeline can start before 19MB of weights land.
        x_tiles = {}

        def issue_x(sb):
            xb = io_in.tile([128, S, D], BF, name="xb", tag="xb")
            xv = x[sb * SB:(sb + 1) * SB].rearrange("(s p) d -> p s d", p=128)
            nc.sync.dma_start(out=xb, in_=xv)
            xt = work.tile([128, KC1, SB], BF, name="xt", tag="xt")
            for k in range(KC1):
                # sb0's chunks are on the critical path: stripe them across
                # both HWDGE rings (the sync ring is ~3x faster).
                eng = nc.sync if (sb == 0 and k < 4) else nc.scalar
                eng.dma_start(
                    out=xt[:, k, :],
                    in_=xT[k * 128:(k + 1) * 128, sb * SB:(sb + 1) * SB],
                )
            x_tiles[sb] = (xb, xt)

        issue_x(0)

        # Resident weights: [p, k, n] with contraction index = k*128 + p.
        # W1 lands in (value-block, gate-block) column pairs so the first
        # GEMM1 chunks can start ~10us in instead of waiting for 12.6MB.
        w1s = weights.tile([128, KC1, H2], BF)
        for nb in range(4):
            for base in (0, H):
                c0, c1 = base + nb * 512, base + (nb + 1) * 512
                for k in range(KC1):
                    nc.sync.dma_start(out=w1s[:, k, c0:c1],
                                      in_=w1[k * 128:(k + 1) * 128, c0:c1])
        w2s = weights.tile([128, KC2, D], BF)
        for k in range(KC2):
            nc.sync.dma_start(out=w2s[:, k, :], in_=w2[k * 128:(k + 1) * 128, :])

        ident = weights.tile([128, 128], F32)
        make_identity(nc, ident)
        # sel[s, s*128+q] = 1: selector for the partition-broadcast matmul
        sels = weights.tile([S, SB], F32)
        nc.sync.dma_start(out=sels, in_=sel[:, :])
        bias0 = weights.tile([128, 1], F32)
        nc.vector.memset(bias0, 0.0)

        for sb in range(nsb):
            if sb + 1 < nsb:
                issue_x(sb + 1)
            xb, xt = x_tiles.pop(sb)

            # --- RMSNorm scale, token-major: ss on ACT, rsqrt on DVE ---
            ssb = small.tile([128, S], F32, name="ssb")
            sq = small.tile([128, D], BF, name="sq")
            for s in range(S):
                nc.scalar.activation(
                    sq, xb[:, s], mybir.ActivationFunctionType.Square,
                    bias=bias0, accum_out=ssb[:, s:s + 1],
                )
            yb = small.tile([128, S], F32, name="yb")
            tb = small.tile([128, S], F32, name="tb")
            # rsqrt seed via the int bit trick: 0x5f3759df - (i >> 1)
            # (written as (i>>1 xor -1) + 0x5f3759df + 1), then 3 Newton steps.
            nc.vector.tensor_scalar(
                out=yb.bitcast(I32), in0=ssb.bitcast(I32),
                scalar1=1, scalar2=-1,
                op0=ALU.logical_shift_right, op1=ALU.bitwise_xor,
            )
            nc.vector.tensor_scalar(
                out=yb.bitcast(I32), in0=yb.bitcast(I32),
                scalar1=0x5F375A60, scalar2=None, op0=ALU.add,
            )
            for _ in range(3):
                nc.vector.tensor_mul(tb, yb, yb)
                nc.vector.tensor_mul(tb, tb, ssb)
                nc.vector.tensor_scalar(
                    out=tb, in0=tb, scalar1=-0.5, scalar2=1.5,
                    op0=ALU.mult, op1=ALU.add,
                )
                nc.vector.tensor_mul(yb, yb, tb)

            # --- broadcast scale across partitions: yb[p,s] -> sc[:,s*128+p]
            yt = psum_yt.tile([S, 128], F32, name="yt", tag="yt", space="PSUM")
            nc.tensor.transpose(yt, yb, ident)
            yrow = small.tile([S, 128], F32, name="yrow")
            nc.vector.tensor_copy(yrow, yt)
            psc = psum_sc.tile([128, SB], F32, name="psc", tag="sc", space="PSUM")
            for s in range(S):
                nc.tensor.matmul(
                    psc[:, s * 128:(s + 1) * 128],
                    lhsT=sels[:, s * 128:(s + 1) * 128],
                    rhs=yrow, start=True, stop=True,
                )
            sc = work.tile([128, SB], F32, name="sc", tag="sc")
            nc.vector.tensor_copy(sc, psc)

            # --- normalize in place in the transposed domain ---
            for k in range(KC1):
                nc.vector.tensor_mul(xt[:, k, :], xt[:, k, :], sc)

            # --- GEMM1 + GEGLU, one value/gate chunk pair at a time.
            # A matmul's fp32 PSUM output cannot cross a 2KB bank, so the
            # 1024-token super-block runs as two 512-column halves. ---
            gbuf = gpool.tile([128, KC2, SB], BF, name="gbuf")
            for m in range(MC):
                for h2 in range(2):
                    cols = slice(h2 * 512, (h2 + 1) * 512)
                    pv = psum_mm.tile([128, 512], F32, name="pv", tag="mm",
                                      space="PSUM")
                    pg = psum_mm.tile([128, 512], F32, name="pg", tag="mm",
                                      space="PSUM")
                    for k in range(KC1):
                        nc.tensor.matmul(
                            pv, lhsT=w1s[:, k, m * 128:(m + 1) * 128],
                            rhs=xt[:, k, cols],
                            start=(k == 0), stop=(k == KC1 - 1),
                        )
                    for k in range(KC1):
                        nc.tensor.matmul(
                            pg, lhsT=w1s[:, k, H + m * 128:H + (m + 1) * 128],
                            rhs=xt[:, k, cols],
                            start=(k == 0), stop=(k == KC1 - 1),
                        )
                    ag = agp.tile([128, 512], F32, name="ag")
                    nc.scalar.activation(
                        ag, pg, mybir.ActivationFunctionType.Gelu, bias=bias0,
                    )
                    nc.vector.tensor_mul(gbuf[:, m, cols], pv, ag)

            # --- GEMM2 with gbuf chunks stationary: PSUM comes out
            # token-major, so results DMA straight out after one copy.
            # d=768 output splits into 512+256 PSUM chains (bank rule). ---
            for mt in range(S):
                ob = obp.tile([128, D], BF, name="ob")
                for d0, d1 in ((0, 512), (512, 768)):
                    po = psum_mm.tile([128, d1 - d0], F32, name="po", tag="mm",
                                      space="PSUM")
                    for k2 in range(KC2):
                        nc.tensor.matmul(
                            po, lhsT=gbuf[:, k2, mt * 128:(mt + 1) * 128],
                            rhs=w2s[:, k2, d0:d1],
                            start=(k2 == 0), stop=(k2 == KC2 - 1),
                        )
                    nc.vector.tensor_copy(ob[:, d0:d1], po)
                nc.gpsimd.dma_start(
                    out=out[sb * SB + mt * 128:sb * SB + (mt + 1) * 128, :],
                    in_=ob,
                )

    nc.finalize()
    return nc


def prepare_in_maps(x, c_fc, c_proj, gamma, mult_bias):
    bf16 = ml_dtypes.bfloat16
    g = (gamma.astype(np.float32) * np.float32(np.sqrt(D)))
    w1_all = (c_fc.astype(np.float32) * g[None, :, None]).astype(bf16)
    w2_all = (c_proj.astype(np.float32)
              * mult_bias.astype(np.float32)[None, :, None]).astype(bf16)
    xs = np.ascontiguousarray(np.transpose(x, (1, 0, 2, 3))).reshape(E, T, D)
    xs = xs.astype(bf16)
    xts = np.ascontiguousarray(np.transpose(xs, (0, 2, 1)))
    sel = np.zeros((S, SB), np.float32)
    for s in range(S):
        sel[s, s * 128:(s + 1) * 128] = 1.0
    return [
        {"x": xs[e], "xT": xts[e], "w1": w1_all[e], "w2": w2_all[e], "sel": sel}
        for e in range(E)
    ]


def run(in_maps, trace: bool = False):
    nc = build_kernel()
    return run_bass_kernel_spmd(
        nc, in_maps, core_ids=list(range(E)), trace=trace,
    )


def kernel(x, c_fc, c_proj, gamma, mult_bias):
    in_maps = prepare_in_maps(x, c_fc, c_proj, gamma, mult_bias)
    res = run(in_maps)
    out = np.empty((E, B, CAP, D), np.float32)
    for e in range(E):
        out[e] = res.results[e]["out"].astype(np.float32).reshape(B, CAP, D)
    return np.ascontiguousarray(out.transpose(1, 0, 2, 3))


# revision 12
# speedup vs baseline: 1.0164x; 1.0123x over previous
"""Expert-parallel MoE GEGLU MLP (RMSNorm -> c_fc -> GEGLU -> c_proj) on 8
Trainium2 NeuronCores.

Sharding: expert-parallel. Core e computes the full MLP for expert e's tokens
(x[:, e] -> [8192, 768]); no collectives. gamma*sqrt(D) is folded into c_fc
and mult_bias into c_proj on the host, so the device kernel computes:

    h   = x / ||x||_2            (per token, fp32 accumulate)
    u   = h @ W1                 (bf16 x bf16 -> fp32 PSUM)
    g   = gelu(u_gate) * u_val   (exact erf gelu on ACT)
    out = g @ W2                 (bf16 x bf16 -> fp32 PSUM)

Layout: tokens stream in super-blocks of 1024. x is loaded twice: once
token-major (for the squared-sum only) and once d-major via the DMA xbar
transpose straight from DRAM. The per-token rsqrt scale is computed
token-major (cheap DVE Newton), moved to a row with one tiny PE transpose,
broadcast across partitions with K=1 matmuls, and applied in place to the
transposed activations. GEMM1 runs with hidden on PSUM partitions and
1024-token moving operands; GEMM2 uses the GEGLU output chunks as the
stationary operand so its PSUM output is already token-major - no output
transposes at all.
"""

from contextlib import ExitStack

import ml_dtypes
import numpy as np

import concourse.bass as bass
import concourse.mybir as mybir
import concourse.tile as tile
from concourse import bacc
from concourse.bass_utils import run_bass_kernel_spmd
from concourse.masks import make_identity

# Problem dims (fixed by the nn_MLP_90795608637901 spec).
B, E, CAP, D = 8, 8, 1024, 768
H = 2048
H2 = 2 * H
T = B * CAP          # tokens per expert (per core) = 8192
SB = 1024            # tokens per super-block
NSB = T // SB        # 8
S = SB // 128        # 8 partition sub-tiles per super-block
KC1 = D // 128       # 6 contraction chunks for GEMM1
MC = H // 128        # 16 value/gate chunk pairs
KC2 = H // 128       # 16 contraction chunks for GEMM2

BF = mybir.dt.bfloat16
F32 = mybir.dt.float32
I32 = mybir.dt.int32
ALU = mybir.AluOpType


def build_kernel(nsb: int = NSB) -> bass.Bass:
    nc = bacc.Bacc("TRN2", target_bir_lowering=False, debug=False)

    t = nsb * SB
    x = nc.declare_dram_parameter("x", [t, D], BF, isOutput=False)
    xT = nc.declare_dram_parameter("xT", [D, t], BF, isOutput=False)
    w1 = nc.declare_dram_parameter("w1", [D, H2], BF, isOutput=False)
    w2 = nc.declare_dram_parameter("w2", [H, D], BF, isOutput=False)
    sel = nc.declare_dram_parameter("sel", [S, SB], F32, isOutput=False)
    out = nc.declare_dram_parameter("out", [t, D], BF, isOutput=True)

    with tile.TileContext(nc) as tc, ExitStack() as ctx:
        weights = ctx.enter_context(tc.tile_pool(name="weights", bufs=1))
        io_in = ctx.enter_context(tc.tile_pool(name="io_in", bufs=2))
        work = ctx.enter_context(tc.tile_pool(name="work", bufs=2))
        gpool = ctx.enter_context(tc.tile_pool(name="gpool", bufs=1))
        small = ctx.enter_context(tc.tile_pool(name="small", bufs=2))
        agp = ctx.enter_context(tc.tile_pool(name="agp", bufs=3))
        obp = ctx.enter_context(tc.tile_pool(name="obp", bufs=3))
        psum_mm = ctx.enter_context(tc.tile_pool(name="psum_mm", bufs=4, space="PSUM"))
        psum_sc = ctx.enter_context(tc.tile_pool(name="psum_sc", bufs=1, space="PSUM"))
        psum_yt = ctx.enter_context(tc.tile_pool(name="psum_yt", bufs=2, space="PSUM"))

        # x DMAs for a super-block; emitted ahead of the weight loads for
        # sb=0 so the PE pipeline can start before 19MB of weights land.
        x_tiles = {}

        def issue_x(sb):
            xb = io_in.tile([128, S, D], BF, name="xb", tag="xb")
            xv = x[sb * SB:(sb + 1) * SB].rearrange("(s p) d -> p s d", p=128)
            nc.sync.dma_start(out=xb, in_=xv)
            xt = work.tile([128, KC1, SB], BF, name="xt", tag="xt")
            for k in range(KC1):
                # sb0's chunks are on the critical path: stripe them across
                # both HWDGE rings (the sync ring is ~3x faster).
                eng = nc.sync if (sb == 0 and k < 4) else nc.scalar
                eng.dma_start(
                    out=xt[:, k, :],
                    in_=xT[k * 128:(k + 1) * 128, sb * SB:(sb + 1) * SB],
                )
            x_tiles[sb] = (xb, xt)

        ident = weights.tile([128, 128], F32)
        make_identity(nc, ident)
        # sel[s, s*128+q] = 1: selector for the partition-broadcast matmul
        sels = weights.tile([S, SB], F32)
        nc.sync.dma_start(out=sels, in_=sel[:, :])
        bias0 = weights.tile([128, 1], F32)
        nc.vector.memset(bias0, 0.0)

        issue_x(0)

        # Resident weights: [p, k, n] with contraction index = k*128 + p.
        # W1 lands in (value-block, gate-block) column pairs so the first
        # GEMM1 chunks can start ~10us in instead of waiting for 12.6MB.
        w1s = weights.tile([128, KC1, H2], BF)
        for nb in range(4):
            for base in (0, H):
                c0, c1 = base + nb * 512, base + (nb + 1) * 512
                for k in range(KC1):
                    nc.sync.dma_start(out=w1s[:, k, c0:c1],
                                      in_=w1[k * 128:(k + 1) * 128, c0:c1])
        w2s = weights.tile([128, KC2, D], BF)
        for k in range(KC2):
            nc.sync.dma_start(out=w2s[:, k, :], in_=w2[k * 128:(k + 1) * 128, :])

        for sb in range(nsb):
            if sb + 1 < nsb:
                issue_x(sb + 1)
            xb, xt = x_tiles.pop(sb)

            # --- RMSNorm scale, token-major: ss on ACT, rsqrt on DVE ---
            ssb = small.tile([128, S], F32, name="ssb")
            sq = small.tile([128, D], BF, name="sq")
            for s in range(S):
                nc.scalar.activation(
                    sq, xb[:, s], mybir.ActivationFunctionType.Square,
                    bias=bias0, accum_out=ssb[:, s:s + 1],
                )
            yb = small.tile([128, S], F32, name="yb")
            tb = small.tile([128, S], F32, name="tb")
            # rsqrt seed via the int bit trick: 0x5f3759df - (i >> 1)
            # (written as (i>>1 xor -1) + 0x5f3759df + 1), then 3 Newton steps.
            nc.vector.tensor_scalar(
                out=yb.bitcast(I32), in0=ssb.bitcast(I32),
                scalar1=1, scalar2=-1,
                op0=ALU.logical_shift_right, op1=ALU.bitwise_xor,
            )
            nc.vector.tensor_scalar(
                out=yb.bitcast(I32), in0=yb.bitcast(I32),
                scalar1=0x5F375A60, scalar2=None, op0=ALU.add,
            )
            for _ in range(3):
                nc.vector.tensor_mul(tb, yb, yb)
                nc.vector.tensor_mul(tb, tb, ssb)
                nc.vector.tensor_scalar(
                    out=tb, in0=tb, scalar1=-0.5, scalar2=1.5,
                    op0=ALU.mult, op1=ALU.add,
                )
                nc.vector.tensor_mul(yb, yb, tb)

            # --- broadcast scale across partitions: yb[p,s] -> sc[:,s*128+p]
            yt = psum_yt.tile([S, 128], F32, name="yt", tag="yt", space="PSUM")
            nc.tensor.transpose(yt, yb, ident)
            yrow = small.tile([S, 128], F32, name="yrow")
            nc.vector.tensor_copy(yrow, yt)
            psc = psum_sc.tile([128, SB], F32, name="psc", tag="sc", space="PSUM")
            for s in range(S):
                nc.tensor.matmul(
                    psc[:, s * 128:(s + 1) * 128],
                    lhsT=sels[:, s * 128:(s + 1) * 128],
                    rhs=yrow, start=True, stop=True,
                )
            sc = work.tile([128, SB], F32, name="sc", tag="sc")
            nc.vector.tensor_copy(sc, psc)

            # --- normalize in place in the transposed domain ---
            for k in range(KC1):
                nc.vector.tensor_mul(xt[:, k, :], xt[:, k, :], sc)

            # --- GEMM1 + GEGLU, one value/gate chunk pair at a time.
            # A matmul's fp32 PSUM output cannot cross a 2KB bank, so the
            # 1024-token super-block runs as two 512-column halves. ---
            gbuf = gpool.tile([128, KC2, SB], BF, name="gbuf")
            for m in range(MC):
                for h2 in range(2):
                    cols = slice(h2 * 512, (h2 + 1) * 512)
                    pv = psum_mm.tile([128, 512], F32, name="pv", tag="mm",
                                      space="PSUM")
                    pg = psum_mm.tile([128, 512], F32, name="pg", tag="mm",
                                      space="PSUM")
                    for k in range(KC1):
                        nc.tensor.matmul(
                            pv, lhsT=w1s[:, k, m * 128:(m + 1) * 128],
                            rhs=xt[:, k, cols],
                            start=(k == 0), stop=(k == KC1 - 1),
                        )
                    for k in range(KC1):
                        nc.tensor.matmul(
                            pg, lhsT=w1s[:, k, H + m * 128:H + (m + 1) * 128],
                            rhs=xt[:, k, cols],
                            start=(k == 0), stop=(k == KC1 - 1),
                        )
                    ag = agp.tile([128, 512], F32, name="ag")
                    nc.scalar.activation(
                        ag, pg, mybir.ActivationFunctionType.Gelu, bias=bias0,
                    )
                    nc.vector.tensor_mul(gbuf[:, m, cols], pv, ag)

            # --- GEMM2 with gbuf chunks stationary: PSUM comes out
            # token-major, so results DMA straight out after one copy.
            # d=768 output splits into 512+256 PSUM chains (bank rule). ---
            for mt in range(S):
                ob = obp.tile([128, D], BF, name="ob")
                for d0, d1 in ((0, 512), (512, 768)):
                    po = psum_mm.tile([128, d1 - d0], F32, name="po", tag="mm",
                                      space="PSUM")
                    for k2 in range(KC2):
                        nc.tensor.matmul(
                            po, lhsT=gbuf[:, k2, mt * 128:(mt + 1) * 128],
                            rhs=w2s[:, k2, d0:d1],
                            start=(k2 == 0), stop=(k2 == KC2 - 1),
                        )
                    nc.vector.tensor_copy(ob[:, d0:d1], po)
                nc.gpsimd.dma_start(
                    out=out[sb * SB + mt * 128:sb * SB + (mt + 1) * 128, :],
                    in_=ob,
                )

    nc.finalize()
    return nc


def prepare_in_maps(x, c_fc, c_proj, gamma, mult_bias):
    bf16 = ml_dtypes.bfloat16
    g = (gamma.astype(np.float32) * np.float32(np.sqrt(D)))
    w1_all = (c_fc.astype(np.float32) * g[None, :, None]).astype(bf16)
    w2_all = (c_proj.astype(np.float32)
              * mult_bias.astype(np.float32)[None, :, None]).astype(bf16)
    xs = np.ascontiguousarray(np.transpose(x, (1, 0, 2, 3))).reshape(E, T, D)
    xs = xs.astype(bf16)
    xts = np.ascontiguousarray(np.transpose(xs, (0, 2, 1)))
    sel = np.zeros((S, SB), np.float32)
    for s in range(S):
        sel[s, s * 128:(s + 1) * 128] = 1.0
    return [
        {"x": xs[e], "xT": xts[e], "w1": w1_all[e], "w2": w2_all[e], "sel": sel}
        for e in range(E)
    ]


def run(in_maps, trace: bool = False):
    nc = build_kernel()
    return run_bass_kernel_spmd(
        nc, in_maps, core_ids=list(range(E)), trace=trace,
    )


def kernel(x, c_fc, c_proj, gamma, mult_bias):
    in_maps = prepare_in_maps(x, c_fc, c_proj, gamma, mult_bias)
    res = run(in_maps)
    out = np.empty((E, B, CAP, D), np.float32)
    for e in range(E):
        out[e] = res.results[e]["out"].astype(np.float32).reshape(B, CAP, D)
    return np.ascontiguousarray(out.transpose(1, 0, 2, 3))


# revision 14
# speedup vs baseline: 1.0448x; 1.0279x over previous
"""Expert-parallel MoE GEGLU MLP (RMSNorm -> c_fc -> GEGLU -> c_proj) on 8
Trainium2 NeuronCores.

Sharding: expert-parallel. Core e computes the full MLP for expert e's tokens
(x[:, e] -> [8192, 768]); no collectives. gamma*sqrt(D) is folded into c_fc
and mult_bias into c_proj on the host, so the device kernel computes:

    h   = x / ||x||_2            (per token, fp32 accumulate)
    u   = h @ W1                 (bf16 x bf16 -> fp32 PSUM)
    g   = gelu(u_gate) * u_val   (exact erf gelu on ACT)
    out = g @ W2                 (bf16 x bf16 -> fp32 PSUM)

Layout: tokens stream in super-blocks of 1024. x is loaded twice: once
token-major (for the squared-sum only) and once d-major via the DMA xbar
transpose straight from DRAM. The per-token rsqrt scale is computed
token-major (cheap DVE Newton), moved to a row with one tiny PE transpose,
broadcast across partitions with K=1 matmuls, and applied in place to the
transposed activations. GEMM1 runs with hidden on PSUM partitions and
1024-token moving operands; GEMM2 uses the GEGLU output chunks as the
stationary operand so its PSUM output is already token-major - no output
transposes at all.
"""

from contextlib import ExitStack

import ml_dtypes
import numpy as np

import concourse.bass as bass
import concourse.mybir as mybir
import concourse.tile as tile
from concourse import bacc
from concourse.bass_utils import run_bass_kernel_spmd
from concourse.masks import make_identity

# Problem dims (fixed by the nn_MLP_90795608637901 spec).
B, E, CAP, D = 8, 8, 1024, 768
H = 2048
H2 = 2 * H
T = B * CAP          # tokens per expert (per core) = 8192
SB = 1024            # tokens per super-block
NSB = T // SB        # 8
S = SB // 128        # 8 partition sub-tiles per super-block
KC1 = D // 128       # 6 contraction chunks for GEMM1
MC = H // 128        # 16 value/gate chunk pairs
KC2 = H // 128       # 16 contraction chunks for GEMM2

BF = mybir.dt.bfloat16
F32 = mybir.dt.float32
I32 = mybir.dt.int32
ALU = mybir.AluOpType


def build_kernel(nsb: int = NSB) -> bass.Bass:
    nc = bacc.Bacc("TRN2", target_bir_lowering=False, debug=False)

    t = nsb * SB
    x = nc.declare_dram_parameter("x", [t, D], BF, isOutput=False)
    xT = nc.declare_dram_parameter("xT", [D, t], BF, isOutput=False)
    w1 = nc.declare_dram_parameter("w1", [D, H2], BF, isOutput=False)
    w2 = nc.declare_dram_parameter("w2", [H, D], BF, isOutput=False)
    sel = nc.declare_dram_parameter("sel", [S, SB], F32, isOutput=False)
    out = nc.declare_dram_parameter("out", [t, D], BF, isOutput=True)

    with tile.TileContext(nc) as tc, ExitStack() as ctx:
        weights = ctx.enter_context(tc.tile_pool(name="weights", bufs=1))
        io_in = ctx.enter_context(tc.tile_pool(name="io_in", bufs=2))
        work = ctx.enter_context(tc.tile_pool(name="work", bufs=2))
        gpool = ctx.enter_context(tc.tile_pool(name="gpool", bufs=1))
        small = ctx.enter_context(tc.tile_pool(name="small", bufs=2))
        agp = ctx.enter_context(tc.tile_pool(name="agp", bufs=3))
        obp = ctx.enter_context(tc.tile_pool(name="obp", bufs=3))
        psum_mm = ctx.enter_context(tc.tile_pool(name="psum_mm", bufs=4, space="PSUM"))
        psum_sc = ctx.enter_context(tc.tile_pool(name="psum_sc", bufs=1, space="PSUM"))
        psum_yt = ctx.enter_context(tc.tile_pool(name="psum_yt", bufs=2, space="PSUM"))

        # x DMAs for a super-block; emitted ahead of the weight loads for
        # sb=0 so the PE pipeline can start before 19MB of weights land.
        x_tiles = {}

        def issue_x(sb):
            xb = io_in.tile([128, S, D], BF, name="xb", tag="xb")
            xv = x[sb * SB:(sb + 1) * SB].rearrange("(s p) d -> p s d", p=128)
            nc.sync.dma_start(out=xb, in_=xv)
            xt = work.tile([128, KC1, SB], BF, name="xt", tag="xt")
            for k in range(KC1):
                nc.scalar.dma_start(
                    out=xt[:, k, :],
                    in_=xT[k * 128:(k + 1) * 128, sb * SB:(sb + 1) * SB],
                )
            x_tiles[sb] = (xb, xt)

        ident = weights.tile([128, 128], F32)
        make_identity(nc, ident)
        # sel[s, s*128+q] = 1: selector for the partition-broadcast matmul
        sels = weights.tile([S, SB], F32)
        nc.sync.dma_start(out=sels, in_=sel[:, :])
        bias0 = weights.tile([128, 1], F32)
        nc.vector.memset(bias0, 0.0)

        # Startup-ordered sync-ring head: xb0, first W1 column pair, then
        # xt0 — exactly what the first GEMM1 chains consume, in that order.
        # W1 lands in (value-block, gate-block) column pairs so the first
        # GEMM1 chunks can start ~10us in instead of waiting for 12.6MB.
        w1s = weights.tile([128, KC1, H2], BF)

        def w1_pair(nb):
            for base in (0, H):
                c0, c1 = base + nb * 512, base + (nb + 1) * 512
                for k in range(KC1):
                    nc.sync.dma_start(out=w1s[:, k, c0:c1],
                                      in_=w1[k * 128:(k + 1) * 128, c0:c1])

        xb0 = io_in.tile([128, S, D], BF, name="xb", tag="xb")
        nc.sync.dma_start(out=xb0, in_=x[0:SB].rearrange("(s p) d -> p s d", p=128))
        w1_pair(0)
        xt0 = work.tile([128, KC1, SB], BF, name="xt", tag="xt")
        for k in range(KC1):
            eng = nc.sync if k < 4 else nc.scalar
            eng.dma_start(out=xt0[:, k, :], in_=xT[k * 128:(k + 1) * 128, 0:SB])
        x_tiles[0] = (xb0, xt0)
        for nb in range(1, 4):
            w1_pair(nb)
        w2s = weights.tile([128, KC2, D], BF)
        for k in range(KC2):
            nc.sync.dma_start(out=w2s[:, k, :], in_=w2[k * 128:(k + 1) * 128, :])

        normed = {}

        def norm_pipeline(sb):
            xb, xt = x_tiles.pop(sb)
            # --- RMSNorm scale, token-major: ss on ACT, rsqrt on DVE ---
            ssb = small.tile([128, S], F32, name="ssb")
            sq = small.tile([128, D], BF, name="sq")
            for s in range(S):
                nc.scalar.activation(
                    sq, xb[:, s], mybir.ActivationFunctionType.Square,
                    bias=bias0, accum_out=ssb[:, s:s + 1],
                )
            yb = small.tile([128, S], F32, name="yb")
            tb = small.tile([128, S], F32, name="tb")
            # rsqrt seed via the int bit trick: 0x5f3759df - (i >> 1)
            # (written as (i>>1 xor -1) + 0x5f3759df + 1), then 3 Newton steps.
            nc.vector.tensor_scalar(
                out=yb.bitcast(I32), in0=ssb.bitcast(I32),
                scalar1=1, scalar2=-1,
                op0=ALU.logical_shift_right, op1=ALU.bitwise_xor,
            )
            nc.vector.tensor_scalar(
                out=yb.bitcast(I32), in0=yb.bitcast(I32),
                scalar1=0x5F375A60, scalar2=None, op0=ALU.add,
            )
            for _ in range(3):
                nc.vector.tensor_mul(tb, yb, yb)
                nc.vector.tensor_mul(tb, tb, ssb)
                nc.vector.tensor_scalar(
                    out=tb, in0=tb, scalar1=-0.5, scalar2=1.5,
                    op0=ALU.mult, op1=ALU.add,
                )
                nc.vector.tensor_mul(yb, yb, tb)

            # --- broadcast scale across partitions: yb[p,s] -> sc[:,s*128+p]
            yt = psum_yt.tile([S, 128], F32, name="yt", tag="yt", space="PSUM")
            nc.tensor.transpose(yt, yb, ident)
            yrow = small.tile([S, 128], F32, name="yrow")
            nc.vector.tensor_copy(yrow, yt)
            psc = psum_sc.tile([128, SB], F32, name="psc", tag="sc", space="PSUM")
            for s in range(S):
                nc.tensor.matmul(
                    psc[:, s * 128:(s + 1) * 128],
                    lhsT=sels[:, s * 128:(s + 1) * 128],
                    rhs=yrow, start=True, stop=True,
                )
            sc = work.tile([128, SB], F32, name="sc", tag="sc")
            nc.vector.tensor_copy(sc, psc)

            # --- normalize in place in the transposed domain ---
            for k in range(KC1):
                nc.vector.tensor_mul(xt[:, k, :], xt[:, k, :], sc)
            normed[sb] = xt

        norm_pipeline(0)
        for sb in range(nsb):
            if sb + 1 < nsb:
                issue_x(sb + 1)
            xt = normed.pop(sb)

            # --- GEMM1 + GEGLU, one value/gate chunk pair at a time.
            # A matmul's fp32 PSUM output cannot cross a 2KB bank, so the
            # 1024-token super-block runs as two 512-column halves. ---
            gbuf = gpool.tile([128, KC2, SB], BF, name="gbuf")
            for m in range(MC):
                for h2 in range(2):
                    cols = slice(h2 * 512, (h2 + 1) * 512)
                    pv = psum_mm.tile([128, 512], F32, name="pv", tag="mm",
                                      space="PSUM")
                    pg = psum_mm.tile([128, 512], F32, name="pg", tag="mm",
                                      space="PSUM")
                    for k in range(KC1):
                        nc.tensor.matmul(
                            pv, lhsT=w1s[:, k, m * 128:(m + 1) * 128],
                            rhs=xt[:, k, cols],
                            start=(k == 0), stop=(k == KC1 - 1),
                        )
                    for k in range(KC1):
                        nc.tensor.matmul(
                            pg, lhsT=w1s[:, k, H + m * 128:H + (m + 1) * 128],
                            rhs=xt[:, k, cols],
                            start=(k == 0), stop=(k == KC1 - 1),
                        )
                    ag = agp.tile([128, 512], F32, name="ag")
                    nc.scalar.activation(
                        ag, pg, mybir.ActivationFunctionType.Gelu, bias=bias0,
                    )
                    nc.vector.tensor_mul(gbuf[:, m, cols], pv, ag)

            if sb + 1 < nsb:
                norm_pipeline(sb + 1)

            # --- GEMM2 with gbuf chunks stationary: PSUM comes out
            # token-major, so results DMA straight out after one copy.
            # d=768 output splits into 512+256 PSUM chains (bank rule). ---
            for mt in range(S):
                ob = obp.tile([128, D], BF, name="ob")
                for d0, d1 in ((0, 512), (512, 768)):
                    po = psum_mm.tile([128, d1 - d0], F32, name="po", tag="mm",
                                      space="PSUM")
                    for k2 in range(KC2):
                        nc.tensor.matmul(
                            po, lhsT=gbuf[:, k2, mt * 128:(mt + 1) * 128],
                            rhs=w2s[:, k2, d0:d1],
                            start=(k2 == 0), stop=(k2 == KC2 - 1),
                        )
                    nc.vector.tensor_copy(ob[:, d0:d1], po)
                nc.gpsimd.dma_start(
                    out=out[sb * SB + mt * 128:sb * SB + (mt + 1) * 128, :],
                    in_=ob,
                )

    nc.finalize()
    return nc


def prepare_in_maps(x, c_fc, c_proj, gamma, mult_bias):
    bf16 = ml_dtypes.bfloat16
    g = (gamma.astype(np.float32) * np.float32(np.sqrt(D)))
    w1_all = (c_fc.astype(np.float32) * g[None, :, None]).astype(bf16)
    w2_all = (c_proj.astype(np.float32)
              * mult_bias.astype(np.float32)[None, :, None]).astype(bf16)
    xs = np.ascontiguousarray(np.transpose(x, (1, 0, 2, 3))).reshape(E, T, D)
    xs = xs.astype(bf16)
    xts = np.ascontiguousarray(np.transpose(xs, (0, 2, 1)))
    sel = np.zeros((S, SB), np.float32)
    for s in range(S):
        sel[s, s * 128:(s + 1) * 128] = 1.0
    return [
        {"x": xs[e], "xT": xts[e], "w1": w1_all[e], "w2": w2_all[e], "sel": sel}
        for e in range(E)
    ]


def run(in_maps, trace: bool = False):
    nc = build_kernel()
    return run_bass_kernel_spmd(
        nc, in_maps, core_ids=list(range(E)), trace=trace,
    )


def kernel(x, c_fc, c_proj, gamma, mult_bias):
    in_maps = prepare_in_maps(x, c_fc, c_proj, gamma, mult_bias)
    res = run(in_maps)
    out = np.empty((E, B, CAP, D), np.float32)
    for e in range(E):
        out[e] = res.results[e]["out"].astype(np.float32).reshape(B, CAP, D)
    return np.ascontiguousarray(out.transpose(1, 0, 2, 3))


# revision 15
# speedup vs baseline: 1.0515x; 1.0064x over previous
"""Expert-parallel MoE GEGLU MLP (RMSNorm -> c_fc -> GEGLU -> c_proj) on 8
Trainium2 NeuronCores.

Sharding: expert-parallel. Core e computes the full MLP for expert e's tokens
(x[:, e] -> [8192, 768]); no collectives. gamma*sqrt(D) is folded into c_fc
and mult_bias into c_proj on the host, so the device kernel computes:

    h   = x / ||x||_2            (per token, fp32 accumulate)
    u   = h @ W1                 (bf16 x bf16 -> fp32 PSUM)
    g   = gelu(u_gate) * u_val   (exact erf gelu on ACT)
    out = g @ W2                 (bf16 x bf16 -> fp32 PSUM)

Layout: tokens stream in super-blocks of 1024. x is loaded twice: once
token-major (for the squared-sum only) and once d-major via the DMA xbar
transpose straight from DRAM. The per-token rsqrt scale is computed
token-major (cheap DVE Newton), moved to a row with one tiny PE transpose,
broadcast across partitions with K=1 matmuls, and applied in place to the
transposed activations. GEMM1 runs with hidden on PSUM partitions and
1024-token moving operands; GEMM2 uses the GEGLU output chunks as the
stationary operand so its PSUM output is already token-major - no output
transposes at all.
"""

from contextlib import ExitStack

import ml_dtypes
import numpy as np

import concourse.bass as bass
import concourse.mybir as mybir
import concourse.tile as tile
from concourse import bacc
from concourse.bass_utils import run_bass_kernel_spmd
from concourse.masks import make_identity

# Problem dims (fixed by the nn_MLP_90795608637901 spec).
B, E, CAP, D = 8, 8, 1024, 768
H = 2048
H2 = 2 * H
T = B * CAP          # tokens per expert (per core) = 8192
SB = 1024            # tokens per super-block
NSB = T // SB        # 8
S = SB // 128        # 8 partition sub-tiles per super-block
KC1 = D // 128       # 6 contraction chunks for GEMM1
MC = H // 128        # 16 value/gate chunk pairs
KC2 = H // 128       # 16 contraction chunks for GEMM2

BF = mybir.dt.bfloat16
F32 = mybir.dt.float32
I32 = mybir.dt.int32
ALU = mybir.AluOpType


def build_kernel(nsb: int = NSB) -> bass.Bass:
    nc = bacc.Bacc("TRN2", target_bir_lowering=False, debug=False)

    t = nsb * SB
    x = nc.declare_dram_parameter("x", [t, D], BF, isOutput=False)
    xT = nc.declare_dram_parameter("xT", [D, t], BF, isOutput=False)
    w1 = nc.declare_dram_parameter("w1", [D, H2], BF, isOutput=False)
    w2 = nc.declare_dram_parameter("w2", [H, D], BF, isOutput=False)
    sel = nc.declare_dram_parameter("sel", [S, SB], F32, isOutput=False)
    out = nc.declare_dram_parameter("out", [t, D], BF, isOutput=True)

    with tile.TileContext(nc) as tc, ExitStack() as ctx:
        weights = ctx.enter_context(tc.tile_pool(name="weights", bufs=1))
        io_in = ctx.enter_context(tc.tile_pool(name="io_in", bufs=2))
        work = ctx.enter_context(tc.tile_pool(name="work", bufs=2))
        gpool = ctx.enter_context(tc.tile_pool(name="gpool", bufs=1))
        small = ctx.enter_context(tc.tile_pool(name="small", bufs=2))
        agp = ctx.enter_context(tc.tile_pool(name="agp", bufs=3))
        obp = ctx.enter_context(tc.tile_pool(name="obp", bufs=3))
        psum_mm = ctx.enter_context(tc.tile_pool(name="psum_mm", bufs=5, space="PSUM"))
        psum_sc = ctx.enter_context(tc.tile_pool(name="psum_sc", bufs=1, space="PSUM"))
        psum_yt = ctx.enter_context(tc.tile_pool(name="psum_yt", bufs=1, space="PSUM"))

        # x DMAs for a super-block; emitted ahead of the weight loads for
        # sb=0 so the PE pipeline can start before 19MB of weights land.
        x_tiles = {}

        def issue_x(sb):
            xb = io_in.tile([128, S, D], BF, name="xb", tag="xb")
            xv = x[sb * SB:(sb + 1) * SB].rearrange("(s p) d -> p s d", p=128)
            nc.sync.dma_start(out=xb, in_=xv)
            xt = work.tile([128, KC1, SB], BF, name="xt", tag="xt")
            for k in range(KC1):
                nc.scalar.dma_start(
                    out=xt[:, k, :],
                    in_=xT[k * 128:(k + 1) * 128, sb * SB:(sb + 1) * SB],
                )
            x_tiles[sb] = (xb, xt)

        ident = weights.tile([128, 128], F32)
        make_identity(nc, ident)
        # sel[s, s*128+q] = 1: selector for the partition-broadcast matmul
        sels = weights.tile([S, SB], F32)
        nc.sync.dma_start(out=sels, in_=sel[:, :])
        bias0 = weights.tile([128, 1], F32)
        nc.vector.memset(bias0, 0.0)

        # Startup-ordered sync-ring head: xb0, first W1 column pair, then
        # xt0 — exactly what the first GEMM1 chains consume, in that order.
        # W1 lands in (value-block, gate-block) column pairs so the first
        # GEMM1 chunks can start ~10us in instead of waiting for 12.6MB.
        w1s = weights.tile([128, KC1, H2], BF)

        def w1_pair(nb):
            for base in (0, H):
                c0, c1 = base + nb * 512, base + (nb + 1) * 512
                for k in range(KC1):
                    nc.sync.dma_start(out=w1s[:, k, c0:c1],
                                      in_=w1[k * 128:(k + 1) * 128, c0:c1])

        xb0 = io_in.tile([128, S, D], BF, name="xb", tag="xb")
        nc.sync.dma_start(out=xb0, in_=x[0:SB].rearrange("(s p) d -> p s d", p=128))
        w1_pair(0)
        xt0 = work.tile([128, KC1, SB], BF, name="xt", tag="xt")
        for k in range(KC1):
            eng = nc.sync if k < 4 else nc.scalar
            eng.dma_start(out=xt0[:, k, :], in_=xT[k * 128:(k + 1) * 128, 0:SB])
        x_tiles[0] = (xb0, xt0)
        for nb in range(1, 4):
            w1_pair(nb)
        w2s = weights.tile([128, KC2, D], BF)
        for k in range(KC2):
            nc.sync.dma_start(out=w2s[:, k, :], in_=w2[k * 128:(k + 1) * 128, :])

        normed = {}

        def norm_pipeline(sb):
            xb, xt = x_tiles.pop(sb)
            # --- RMSNorm scale, token-major: ss on ACT, rsqrt on DVE ---
            ssb = small.tile([128, S], F32, name="ssb")
            sq = small.tile([128, D], BF, name="sq")
            for s in range(S):
                nc.scalar.activation(
                    sq, xb[:, s], mybir.ActivationFunctionType.Square,
                    bias=bias0, accum_out=ssb[:, s:s + 1],
                )
            yb = small.tile([128, S], F32, name="yb")
            tb = small.tile([128, S], F32, name="tb")
            # rsqrt seed via the int bit trick: 0x5f3759df - (i >> 1)
            # (written as (i>>1 xor -1) + 0x5f3759df + 1), then 3 Newton steps.
            nc.vector.tensor_scalar(
                out=yb.bitcast(I32), in0=ssb.bitcast(I32),
                scalar1=1, scalar2=-1,
                op0=ALU.logical_shift_right, op1=ALU.bitwise_xor,
            )
            nc.vector.tensor_scalar(
                out=yb.bitcast(I32), in0=yb.bitcast(I32),
                scalar1=0x5F375A60, scalar2=None, op0=ALU.add,
            )
            for _ in range(3):
                nc.vector.tensor_mul(tb, yb, yb)
                nc.vector.tensor_mul(tb, tb, ssb)
                nc.vector.tensor_scalar(
                    out=tb, in0=tb, scalar1=-0.5, scalar2=1.5,
                    op0=ALU.mult, op1=ALU.add,
                )
                nc.vector.tensor_mul(yb, yb, tb)

            # --- broadcast scale across partitions: yb[p,s] -> sc[:,s*128+p]
            yt = psum_yt.tile([S, 128], F32, name="yt", tag="yt", space="PSUM")
            nc.tensor.transpose(yt, yb, ident)
            yrow = small.tile([S, 128], F32, name="yrow")
            nc.vector.tensor_copy(yrow, yt)
            psc = psum_sc.tile([128, SB], F32, name="psc", tag="sc", space="PSUM")
            for s in range(S):
                nc.tensor.matmul(
                    psc[:, s * 128:(s + 1) * 128],
                    lhsT=sels[:, s * 128:(s + 1) * 128],
                    rhs=yrow, start=True, stop=True,
                )
            sc = work.tile([128, SB], F32, name="sc", tag="sc")
            nc.vector.tensor_copy(sc, psc)

            # --- normalize in place in the transposed domain ---
            for k in range(KC1):
                nc.vector.tensor_mul(xt[:, k, :], xt[:, k, :], sc)
            normed[sb] = xt

        norm_pipeline(0)
        for sb in range(nsb):
            if sb + 1 < nsb:
                issue_x(sb + 1)
            xt = normed.pop(sb)

            # --- GEMM1 + GEGLU, one value/gate chunk pair at a time.
            # A matmul's fp32 PSUM output cannot cross a 2KB bank, so the
            # 1024-token super-block runs as two 512-column halves. ---
            gbuf = gpool.tile([128, KC2, SB], BF, name="gbuf")
            for m in range(MC):
                for h2 in range(2):
                    cols = slice(h2 * 512, (h2 + 1) * 512)
                    pv = psum_mm.tile([128, 512], F32, name="pv", tag="mm",
                                      space="PSUM")
                    pg = psum_mm.tile([128, 512], F32, name="pg", tag="mm",
                                      space="PSUM")
                    for k in range(KC1):
                        nc.tensor.matmul(
                            pv, lhsT=w1s[:, k, m * 128:(m + 1) * 128],
                            rhs=xt[:, k, cols],
                            start=(k == 0), stop=(k == KC1 - 1),
                        )
                    for k in range(KC1):
                        nc.tensor.matmul(
                            pg, lhsT=w1s[:, k, H + m * 128:H + (m + 1) * 128],
                            rhs=xt[:, k, cols],
                            start=(k == 0), stop=(k == KC1 - 1),
                        )
                    ag = agp.tile([128, 512], F32, name="ag")
                    nc.scalar.activation(
                        ag, pg, mybir.ActivationFunctionType.Gelu, bias=bias0,
                    )
                    nc.vector.tensor_mul(gbuf[:, m, cols], pv, ag)

            if sb + 1 < nsb:
                norm_pipeline(sb + 1)

            # --- GEMM2 with gbuf chunks stationary: PSUM comes out
            # token-major, so results DMA straight out after one copy.
            # d=768 output splits into 512+256 PSUM chains (bank rule). ---
            for mt in range(S):
                ob = obp.tile([128, D], BF, name="ob")
                for d0, d1 in ((0, 512), (512, 768)):
                    po = psum_mm.tile([128, d1 - d0], F32, name="po", tag="mm",
                                      space="PSUM")
                    for k2 in range(KC2):
                        nc.tensor.matmul(
                            po, lhsT=gbuf[:, k2, mt * 128:(mt + 1) * 128],
                            rhs=w2s[:, k2, d0:d1],
                            start=(k2 == 0), stop=(k2 == KC2 - 1),
                        )
                    nc.vector.tensor_copy(ob[:, d0:d1], po)
                nc.gpsimd.dma_start(
                    out=out[sb * SB + mt * 128:sb * SB + (mt + 1) * 128, :],
                    in_=ob,
                )

    nc.finalize()
    return nc


def prepare_in_maps(x, c_fc, c_proj, gamma, mult_bias):
    bf16 = ml_dtypes.bfloat16
    g = (gamma.astype(np.float32) * np.float32(np.sqrt(D)))
    w1_all = (c_fc.astype(np.float32) * g[None, :, None]).astype(bf16)
    w2_all = (c_proj.astype(np.float32)
              * mult_bias.astype(np.float32)[None, :, None]).astype(bf16)
    xs = np.ascontiguousarray(np.transpose(x, (1, 0, 2, 3))).reshape(E, T, D)
    xs = xs.astype(bf16)
    xts = np.ascontiguousarray(np.transpose(xs, (0, 2, 1)))
    sel = np.zeros((S, SB), np.float32)
    for s in range(S):
        sel[s, s * 128:(s + 1) * 128] = 1.0
    return [
        {"x": xs[e], "xT": xts[e], "w1": w1_all[e], "w2": w2_all[e], "sel": sel}
        for e in range(E)
    ]


def run(in_maps, trace: bool = False):
    nc = build_kernel()
    return run_bass_kernel_spmd(
        nc, in_maps, core_ids=list(range(E)), trace=trace,
    )


def kernel(x, c_fc, c_proj, gamma, mult_bias):
    in_maps = prepare_in_maps(x, c_fc, c_proj, gamma, mult_bias)
    res = run(in_maps)
    out = np.empty((E, B, CAP, D), np.float32)
    for e in range(E):
        out[e] = res.results[e]["out"].astype(np.float32).reshape(B, CAP, D)
    return np.ascontiguousarray(out.transpose(1, 0, 2, 3))
